# revision 14
# baseline (speedup 1.0000x reference)
"""Single-head causal attention (B=4, L=2048, D=1024) on 8 TRN2 NeuronCores.

Sharding: data-parallel over batch (4) x query-halves (2) = 8 cores.
Each core computes K/V projections over its batch's full 2048 keys and
Q/scores/AV over its 1024-query half.

Device-side layout (per core, all bf16 matmul operands, f32 PSUM accum):
  - scores computed TRANSPOSED: sT[k_tile(128 part), q(1024 free)] so the
    pad mask (a per-key quantity) is a per-partition tensor_scalar and the
    softmax normalizer Z comes from a ones-column appended to V (no
    partition reductions, no P transposes anywhere).
  - masked_fill semantics (exact, incl. degenerate all-masked rows):
      cmp[k,q]  = (iota_q >= thresh[k]) * padkeep[k]   in {0,1}
      sM        = (s_raw + 960) * cmp                  # 0 where masked
      E         = exp(sM/32 - 30)                      # = exp(s/32) kept,
                                                       #   exp(-30) masked
    All-masked rows get exactly-uniform weights, matching jax softmax of
    an all -10000 row.
  - out[q,m] = (E^T V)[q,m] / Z[q], Z from the ones column of V.
"""
import sys

if "/opt/trn_rl_repo" not in sys.path:
    sys.path.insert(0, "/opt/trn_rl_repo")

import numpy as np
import ml_dtypes

import concourse.bass as bass
import concourse.mybir as mybir
from concourse import bacc, tile
from concourse import bass_utils

F32 = mybir.dt.float32
BF16 = mybir.dt.bfloat16
BF16NP = ml_dtypes.bfloat16

B, L, D = 4, 2048, 1024
NQ = L // 2          # queries per core
NKT = L // 128       # 16 k-tiles
NMC = D // 128       # 8 contraction/model chunks
NQT = NQ // 128      # 8 q-tiles per core

_NC_CACHE = None


def _build_nc():
    nc = bacc.Bacc(None, target_bir_lowering=False)

    xt_d = nc.dram_tensor("xt", [128, NMC, L], BF16, kind="ExternalInput")
    xq_d = nc.dram_tensor("xq", [128, NMC, NQ], BF16, kind="ExternalInput")
    wq_d = nc.dram_tensor("wq", [128, NMC, D], BF16, kind="ExternalInput")
    wk_d = nc.dram_tensor("wk", [128, NMC, D], BF16, kind="ExternalInput")
    wv_d = nc.dram_tensor("wv", [128, NMC, D], BF16, kind="ExternalInput")
    padk_d = nc.dram_tensor("padk", [128, NKT], F32, kind="ExternalInput")
    thr_d = nc.dram_tensor("thr", [128, NKT], F32, kind="ExternalInput")
    out_d = nc.dram_tensor("out", [NQ, D], F32, kind="ExternalOutput")

    AL = mybir.AluOpType
    AF = mybir.ActivationFunctionType

    with tile.TileContext(nc) as tc:
        with (
            tc.tile_pool(name="c", bufs=1) as cpool,
            tc.tile_pool(name="sh", bufs=1) as spool,
            tc.tile_pool(name="wk_", bufs=3) as wpool,
            tc.tile_pool(name="pp", bufs=3, space="PSUM") as pp,
            tc.tile_pool(name="ppo", bufs=2, space="PSUM") as ppo,
            tc.tile_pool(name="ppz", bufs=1, space="PSUM") as ppz,
        ):
            # persistent tiles
            xt_sb = spool.tile([128, NMC, L], BF16, name="xt_sb", tag="big")
            wk_sb = spool.tile([128, NMC, D], BF16, name="wk_sb", tag="wkqt")
            wv_sb = cpool.tile([128, NMC, D], BF16, name="wv_sb")
            wq_sb = cpool.tile([128, NMC, D], BF16, name="wq_sb")
            xq_sb = cpool.tile([128, NMC, NQ], BF16, name="xq_sb")
            padk_sb = cpool.tile([128, NKT], F32, name="padk_sb")
            thr_sb = cpool.tile([128, NKT], F32, name="thr_sb")
            kT_sb = cpool.tile([128, NMC, L], BF16, name="kT_sb")
            v_sb = cpool.tile([128, NKT, D + 1], BF16, name="v_sb")
            iota_sb = cpool.tile([128, NQ], F32, name="iota_sb")
            bias_sb = cpool.tile([128, 1], F32, name="bias_sb")

            # chunked loads so the first kT matmuls start after chunk 0 lands
            for dd in range(NMC):
                nc.sync.dma_start(wk_sb[:, dd, :], wk_d[:, dd, :])
                nc.sync.dma_start(xt_sb[:, dd, :], xt_d[:, dd, :])
            nc.sync.dma_start(wv_sb[:], wv_d[:])
            nc.sync.dma_start(wq_sb[:], wq_d[:])
            nc.sync.dma_start(xq_sb[:], xq_d[:])
            nc.sync.dma_start(padk_sb[:], padk_d[:])
            nc.sync.dma_start(thr_sb[:], thr_d[:])

            # local q column f (= 128*jl + fi) maps to global q-tile 2*jl + h;
            # iota encodes q_glob - 128*h = 256*jl + fi; thresh data absorbs h.
            nc.gpsimd.iota(
                out=iota_sb[:].rearrange("p (j f) -> p j f", f=128),
                pattern=[[256, NQT], [1, 128]], base=0, channel_multiplier=0,
                allow_small_or_imprecise_dtypes=True,
            )
            nc.vector.memset(bias_sb[:], -30.0)
            nc.vector.memset(v_sb[:, :, D : D + 1], 1.0)

            # ---- Phase 1a: kT[m, tok] = wk.T @ x ----
            # d-outer over two psum tiles: streams xt/wk chunks as they land,
            # and shares each weight load across two matmuls.
            for mi in range(NMC):
                for tbp in range(2):
                    ps0 = pp.tile([128, 512], F32, name="ps")
                    ps1 = pp.tile([128, 512], F32, name="ps")
                    for d in range(NMC):
                        for tb2, psx in ((0, ps0), (1, ps1)):
                            tb = 2 * tbp + tb2
                            nc.tensor.matmul(
                                psx[:],
                                lhsT=wk_sb[:, d, mi * 128 : (mi + 1) * 128],
                                rhs=xt_sb[:, d, tb * 512 : (tb + 1) * 512],
                                start=(d == 0), stop=(d == NMC - 1),
                            )
                    for tb2, psx in ((0, ps0), (1, ps1)):
                        tb = 2 * tbp + tb2
                        nc.scalar.copy(kT_sb[:, mi, tb * 512 : (tb + 1) * 512], psx[:])

            # ---- Phase 1b: V[tok, m] = x @ wv  (lhsT = xT chunk) ----
            for kt in range(NKT):
                ps0 = pp.tile([128, 512], F32, name="ps")
                ps1 = pp.tile([128, 512], F32, name="ps")
                for d in range(NMC):
                    for mb, psx in ((0, ps0), (1, ps1)):
                        nc.tensor.matmul(
                            psx[:],
                            lhsT=xt_sb[:, d, kt * 128 : (kt + 1) * 128],
                            rhs=wv_sb[:, d, mb * 512 : (mb + 1) * 512],
                            start=(d == 0), stop=(d == NMC - 1),
                        )
                for mb, psx in ((0, ps0), (1, ps1)):
                    nc.scalar.copy(v_sb[:, kt, mb * 512 : (mb + 1) * 512], psx[:])

            # ---- Phase 1c: qT[m, q] = wq.T @ xq ----
            qT_sb = spool.tile([128, NMC, NQ], BF16, name="qT_sb", tag="wkqt")
            for mi in range(NMC):
                ps0 = pp.tile([128, 512], F32, name="ps")
                ps1 = pp.tile([128, 512], F32, name="ps")
                for d in range(NMC):
                    for qb, psx in ((0, ps0), (1, ps1)):
                        nc.tensor.matmul(
                            psx[:],
                            lhsT=wq_sb[:, d, mi * 128 : (mi + 1) * 128],
                            rhs=xq_sb[:, d, qb * 512 : (qb + 1) * 512],
                            start=(d == 0), stop=(d == NMC - 1),
                        )
                for qb, psx in ((0, ps0), (1, ps1)):
                    nc.scalar.copy(qT_sb[:, mi, qb * 512 : (qb + 1) * 512], psx[:])

            # ---- Phase 2: scores (transposed) + mask + exp, per k-tile ----
            # Local q-tile jl holds global q-tile 2*jl + h, so k-tile kt is
            # causally live only for jl >= ceil((kt-1)/2): a contiguous tail
            # of the local q axis. Fully-dead (kt, jl) pairs are skipped;
            # the h=0 core's extra tile per jl is killed by cmp data.
            E_sb = spool.tile([128, NKT, NQ], BF16, name="E_sb", tag="big")
            for kt in range(NKT):
                jl0 = kt // 2  # ceil((kt-1)/2): first local q-tile that sees kt
                f0 = jl0 * 128
                cmp = wpool.tile([128, NQ], F32, name="cmp", bufs=2)
                nc.vector.tensor_scalar(
                    out=cmp[:, f0:], in0=iota_sb[:, f0:],
                    scalar1=thr_sb[:, kt : kt + 1], scalar2=padk_sb[:, kt : kt + 1],
                    op0=AL.is_ge, op1=AL.mult,
                )
                s_sb = wpool.tile([128, NQ], F32, name="s_sb", bufs=3)
                blocks = []
                f = f0
                while f < NQ:
                    w = min(512, NQ - f)
                    blocks.append((f, w))
                    f += w
                for (fb, w) in blocks:
                    ps = pp.tile([128, 512], F32, name="ps")
                    for m in range(NMC):
                        nc.tensor.matmul(
                            ps[:, 0:w],
                            lhsT=kT_sb[:, m, kt * 128 : (kt + 1) * 128],
                            rhs=qT_sb[:, m, fb : fb + w],
                            start=(m == 0), stop=(m == NMC - 1),
                        )
                    nc.vector.scalar_tensor_tensor(
                        out=s_sb[:, fb : fb + w], in0=ps[:, 0:w],
                        scalar=960.0, in1=cmp[:, fb : fb + w],
                        op0=AL.add, op1=AL.mult,
                    )
                nc.scalar.activation(
                    out=E_sb[:, kt, f0:], in_=s_sb[:, f0:],
                    func=AF.Exp, bias=bias_sb[:], scale=0.03125,
                )

            # ---- Phase 3: out[q,m] = (E^T @ [V|1])[q,m] / Z[q] ----
            for jl in range(NQT):
                nkt = 2 * jl + 2  # causally-live k-tiles for this q-tile
                po = ppo.tile([128, D], F32, name="po")
                pz = ppz.tile([128, 1], F32, name="pz")
                for kt in range(nkt):
                    lhsT = E_sb[:, kt, jl * 128 : (jl + 1) * 128]
                    nc.tensor.matmul(po[:, 0:512], lhsT=lhsT, rhs=v_sb[:, kt, 0:512],
                                     start=(kt == 0), stop=(kt == nkt - 1))
                    nc.tensor.matmul(po[:, 512:1024], lhsT=lhsT, rhs=v_sb[:, kt, 512:1024],
                                     start=(kt == 0), stop=(kt == nkt - 1))
                    nc.tensor.matmul(pz[:], lhsT=lhsT, rhs=v_sb[:, kt, D : D + 1],
                                     start=(kt == 0), stop=(kt == nkt - 1))
                rec = wpool.tile([128, 1], F32, name="rec", bufs=2)
                nc.vector.reciprocal(rec[:], pz[:])
                o_sb = wpool.tile([128, D], F32, name="o_sb", bufs=3)
                nc.vector.tensor_scalar(
                    out=o_sb[:], in0=po[:], scalar1=rec[:], scalar2=None, op0=AL.mult,
                )
                nc.sync.dma_start(out_d[jl * 128 : (jl + 1) * 128, :], o_sb[:])

    nc.compile()
    return nc


def _chunked(a):
    """[C*128, N] -> [128, C, N] contiguous."""
    c = a.shape[0] // 128
    return np.ascontiguousarray(a.reshape(c, 128, *a.shape[1:]).transpose(1, 0, 2))


def _qsel(h):
    """Global query rows handled by half h: interleaved 128-row q-tiles."""
    return np.concatenate(
        [np.arange(128 * (2 * jl + h), 128 * (2 * jl + h) + 128) for jl in range(NQT)]
    )


def build_in_maps(inputs):
    x = np.asarray(inputs["x"], dtype=np.float32)
    pad = np.asarray(inputs["pad_mask"])
    wq_h = _chunked(np.asarray(inputs["wq"], dtype=np.float32)).astype(BF16NP)
    wk_h = _chunked(np.asarray(inputs["wk"], dtype=np.float32)).astype(BF16NP)
    wv_h = _chunked(np.asarray(inputs["wv"], dtype=np.float32)).astype(BF16NP)

    in_maps = []
    for c in range(8):
        b, h = divmod(c, 2)
        qsel = _qsel(h)
        xtb = _chunked(x[b].T).astype(BF16NP)               # [128, 8, 2048]
        xqb = _chunked(x[b, qsel, :].T).astype(BF16NP)      # [128, 8, 1024]
        keep = (~pad[b]).astype(np.float32)                     # [2048]
        padk = np.ascontiguousarray(keep.reshape(NKT, 128).T)   # [128, 16]
        # keep iff iota (= q_glob - 128h) >= thresh = 128*kt + p - 128*h
        thr = (
            np.add.outer(np.arange(128, dtype=np.float32),
                         128.0 * np.arange(NKT, dtype=np.float32))
            - np.float32(128 * h)
        ).astype(np.float32)                                    # [128, 16]
        in_maps.append({
            "xt": xtb, "xq": xqb, "wq": wq_h, "wk": wk_h, "wv": wv_h,
            "padk": padk, "thr": np.ascontiguousarray(thr),
        })
    return in_maps


def kernel(**inputs):
    global _NC_CACHE
    if _NC_CACHE is None:
        _NC_CACHE = _build_nc()
    nc = _NC_CACHE

    in_maps = build_in_maps(inputs)
    res = bass_utils.run_bass_kernel_spmd(nc, in_maps, core_ids=list(range(8)))
    out = np.empty((B, L, D), dtype=np.float32)
    for b in range(B):
        for h in range(2):
            out[b, _qsel(h)] = res.results[2 * b + h]["out"]
    return out


# revision 24
# speedup vs baseline: 1.3098x; 1.3098x over previous
"""Single-head causal attention (B=4, L=2048, D=1024) on 8 TRN2 NeuronCores.

Sharding: data-parallel over batch (4) x interleaved query-tile halves (2).
Core 2b+h handles batch b and global q-tiles {h, h+2, ..., h+14} (128 rows
each), so the causal loop-trip counts are identical across cores (SPMD) while
still skipping ~44% of the score/AV work.

Compute is fp8e4m3 with DoubleRow matmuls (256-contraction per instruction),
f32 PSUM accumulation. Weights are pre-scaled on host (wq,wk x256; wv x32) to
sit in fp8 range; the exact power-of-2 compensation folds into the exp scale
(2^-21) and the output normalize (x 1/32).

Scores are computed TRANSPOSED: sT[k(128 part), q(free)] so the pad mask (a
per-key quantity) is a per-partition tensor_scalar operand and the softmax
normalizer Z comes from a ones-column appended to V - no partition reductions
or P transposes anywhere. masked_fill is exact:
    cmp[k,q] = (iota_q >= thresh[k]) * padkeep[k]    in {0,1}
    E        = exp(((s' + 960*2^16) * cmp) * 2^-21 - 30)
             = exp(s_raw/32) kept, exp(-30)~=0 masked.
"""
import sys

if "/opt/trn_rl_repo" not in sys.path:
    sys.path.insert(0, "/opt/trn_rl_repo")

import numpy as np
import ml_dtypes

import concourse.bass as bass
import concourse.mybir as mybir
from concourse import bacc, tile
from concourse import bass_utils

F32 = mybir.dt.float32
FP8 = mybir.dt.float8e4
FP8NP = ml_dtypes.float8_e4m3
BF16 = mybir.dt.bfloat16
BF16NP = ml_dtypes.bfloat16

B, L, D = 4, 2048, 1024
NQ = L // 2          # queries per core
NKT = L // 128       # 16 k-tiles
NMC = D // 128       # 8 contraction/model chunks
NQT = NQ // 128      # 8 q-tiles per core
VF = 1025            # v chunk free size (1024 vals + ones col)

SW = 256.0           # host pre-scale for wq, wk (fp8 range)
SV = 1.0             # wv stays bf16: no pre-scale needed
DR = mybir.MatmulPerfMode.DoubleRow

_NC_CACHE = None


def _build_nc():
    nc = bacc.Bacc(None, target_bir_lowering=False)

    xt_d = nc.dram_tensor("xt", [128, NMC, L], FP8, kind="ExternalInput")
    xt16_d = nc.dram_tensor("xt16", [128, NMC, L], BF16, kind="ExternalInput")
    xq_d = nc.dram_tensor("xq", [128, NMC, NQ], FP8, kind="ExternalInput")
    wq_d = nc.dram_tensor("wq", [128, NMC, D], FP8, kind="ExternalInput")
    wk_d = nc.dram_tensor("wk", [128, NMC, D], FP8, kind="ExternalInput")
    wv_d = nc.dram_tensor("wv", [128, NMC, D], BF16, kind="ExternalInput")
    padk_d = nc.dram_tensor("padk", [128, NKT], F32, kind="ExternalInput")
    thr_d = nc.dram_tensor("thr", [128, NKT], F32, kind="ExternalInput")
    out_d = nc.dram_tensor("out", [NQ, D], F32, kind="ExternalOutput")

    AL = mybir.AluOpType
    AF = mybir.ActivationFunctionType

    with tile.TileContext(nc) as tc:
        with (
            tc.tile_pool(name="c", bufs=1) as cpool,
            tc.tile_pool(name="sh", bufs=1) as spool,
            tc.tile_pool(name="wk_", bufs=3) as wpool,
            tc.tile_pool(name="pp", bufs=3, space="PSUM") as pp,
            tc.tile_pool(name="ppo", bufs=2, space="PSUM") as ppo,
            tc.tile_pool(name="ppz", bufs=1, space="PSUM") as ppz,
        ):
            # persistent tiles; xt16 and E share one slot (disjoint lifetimes)
            xt_sb = cpool.tile([128, NMC, L], FP8, name="xt_sb")
            xt16_sb = spool.tile([128, NMC, L], BF16, name="xt16_sb", tag="big")
            wk_sb = cpool.tile([128, NMC, D], FP8, name="wk_sb")
            wv_sb = cpool.tile([128, NMC, D], BF16, name="wv_sb")
            wq_sb = cpool.tile([128, NMC, D], FP8, name="wq_sb")
            xq_sb = cpool.tile([128, NMC, NQ], FP8, name="xq_sb")
            padk_sb = cpool.tile([128, NKT], F32, name="padk_sb")
            thr_sb = cpool.tile([128, NKT], F32, name="thr_sb")
            kT_sb = cpool.tile([128, NMC, L], FP8, name="kT_sb")
            qT_sb = cpool.tile([128, NMC, NQ], FP8, name="qT_sb")
            v_sb = cpool.tile([128, NKT, VF], BF16, name="v_sb")
            iota_sb = cpool.tile([128, NQ], F32, name="iota_sb")
            bias_sb = cpool.tile([128, 1], F32, name="bias_sb")

            # chunked loads so the first kT matmuls start after chunk 0 lands
            for dd in range(NMC):
                nc.sync.dma_start(wk_sb[:, dd, :], wk_d[:, dd, :])
                nc.sync.dma_start(xt_sb[:, dd, :], xt_d[:, dd, :])
            for dd in range(NMC):
                nc.sync.dma_start(xt16_sb[:, dd, :], xt16_d[:, dd, :])
            nc.sync.dma_start(wv_sb[:], wv_d[:])
            nc.sync.dma_start(wq_sb[:], wq_d[:])
            nc.sync.dma_start(xq_sb[:], xq_d[:])
            nc.sync.dma_start(padk_sb[:], padk_d[:])
            nc.sync.dma_start(thr_sb[:], thr_d[:])

            # local q column f (= 128*jl + fi) maps to global q-tile 2*jl + h;
            # iota encodes q_glob - 128*h = 256*jl + fi; thresh data absorbs h.
            nc.gpsimd.iota(
                out=iota_sb[:].rearrange("p (j f) -> p j f", f=128),
                pattern=[[256, NQT], [1, 128]], base=0, channel_multiplier=0,
                allow_small_or_imprecise_dtypes=True,
            )
            nc.vector.memset(bias_sb[:], -30.0)
            nc.vector.memset(v_sb[:, :, D : D + 1], 1.0)

            # ---- Phase 1a: kT[m, tok] = wk.T @ x ----
            for mi in range(NMC):
                for tbp in range(2):
                    ps0 = pp.tile([128, 512], F32, name="ps")
                    ps1 = pp.tile([128, 512], F32, name="ps")
                    for d in range(0, NMC, 2):
                        for tb2, psx in ((0, ps0), (1, ps1)):
                            tb = 2 * tbp + tb2
                            nc.tensor.matmul(
                                psx[:],
                                lhsT=wk_sb[:, d : d + 2, mi * 128 : (mi + 1) * 128],
                                rhs=xt_sb[:, d : d + 2, tb * 512 : (tb + 1) * 512],
                                start=(d == 0), stop=(d == NMC - 2), perf_mode=DR,
                            )
                    for tb2, psx in ((0, ps0), (1, ps1)):
                        tb = 2 * tbp + tb2
                        nc.scalar.copy(kT_sb[:, mi, tb * 512 : (tb + 1) * 512], psx[:])

            # ---- Phase 1b: V[tok, m] = x @ wv in bf16 (value path stays
            # high precision: its quantization error hits the output at
            # full strength, unlike the Q/K path) ----
            for kt in range(NKT):
                ps0 = pp.tile([128, 512], F32, name="ps")
                ps1 = pp.tile([128, 512], F32, name="ps")
                for d in range(NMC):
                    for mb, psx in ((0, ps0), (1, ps1)):
                        nc.tensor.matmul(
                            psx[:],
                            lhsT=xt16_sb[:, d, kt * 128 : (kt + 1) * 128],
                            rhs=wv_sb[:, d, mb * 512 : (mb + 1) * 512],
                            start=(d == 0), stop=(d == NMC - 1),
                        )
                for mb, psx in ((0, ps0), (1, ps1)):
                    nc.scalar.copy(v_sb[:, kt, mb * 512 : (mb + 1) * 512], psx[:])

            # ---- Phase 1c: qT[m, q] = wq.T @ xq ----
            for mi in range(NMC):
                ps0 = pp.tile([128, 512], F32, name="ps")
                ps1 = pp.tile([128, 512], F32, name="ps")
                for d in range(0, NMC, 2):
                    for qb, psx in ((0, ps0), (1, ps1)):
                        nc.tensor.matmul(
                            psx[:],
                            lhsT=wq_sb[:, d : d + 2, mi * 128 : (mi + 1) * 128],
                            rhs=xq_sb[:, d : d + 2, qb * 512 : (qb + 1) * 512],
                            start=(d == 0), stop=(d == NMC - 2), perf_mode=DR,
                        )
                for qb, psx in ((0, ps0), (1, ps1)):
                    nc.scalar.copy(qT_sb[:, mi, qb * 512 : (qb + 1) * 512], psx[:])

            # ---- Phase 2: scores (transposed) + mask + exp, per k-tile ----
            # Local q-tile jl holds global q-tile 2*jl + h, so k-tile kt is
            # causally live only for jl >= kt//2: a contiguous tail of the
            # local q axis. Fully-dead (kt, jl) pairs are skipped; the h=0
            # core's extra tile per jl is killed by cmp data.
            E_sb = spool.tile([128, NKT, NQ], BF16, name="E_sb", tag="big")
            for kt in range(NKT):
                jl0 = kt // 2
                f0 = jl0 * 128
                cmp = wpool.tile([128, NQ], F32, name="cmp", bufs=2)
                nc.vector.tensor_scalar(
                    out=cmp[:, f0:], in0=iota_sb[:, f0:],
                    scalar1=thr_sb[:, kt : kt + 1], scalar2=padk_sb[:, kt : kt + 1],
                    op0=AL.is_ge, op1=AL.mult,
                )
                s_sb = wpool.tile([128, NQ], F32, name="s_sb", bufs=3)
                f = f0
                while f < NQ:
                    w = min(512, NQ - f)
                    ps = pp.tile([128, 512], F32, name="ps")
                    for m in range(0, NMC, 2):
                        nc.tensor.matmul(
                            ps[:, 0:w],
                            lhsT=kT_sb[:, m : m + 2, kt * 128 : (kt + 1) * 128],
                            rhs=qT_sb[:, m : m + 2, f : f + w],
                            start=(m == 0), stop=(m == NMC - 2), perf_mode=DR,
                        )
                    nc.vector.scalar_tensor_tensor(
                        out=s_sb[:, f : f + w], in0=ps[:, 0:w],
                        scalar=62914560.0,  # 960 * 2^16
                        in1=cmp[:, f : f + w],
                        op0=AL.add, op1=AL.mult,
                    )
                    f += w
                nc.scalar.activation(
                    out=E_sb[:, kt, f0:], in_=s_sb[:, f0:],
                    func=AF.Exp, bias=bias_sb[:], scale=2.0 ** -21,
                )

            # ---- Phase 3: out[q,m] = (E^T @ [V|1])[q,m] / Z[q] ----
            for jl in range(NQT):
                nkt = 2 * jl + 2  # causally-live k-tiles for this q-tile
                po = ppo.tile([128, D], F32, name="po")
                pz = ppz.tile([128, 1], F32, name="pz")
                for kt in range(nkt):
                    lhsT = E_sb[:, kt, jl * 128 : (jl + 1) * 128]
                    nc.tensor.matmul(po[:, 0:512], lhsT=lhsT,
                                     rhs=v_sb[:, kt, 0:512],
                                     start=(kt == 0), stop=(kt == nkt - 1))
                    nc.tensor.matmul(po[:, 512:1024], lhsT=lhsT,
                                     rhs=v_sb[:, kt, 512:1024],
                                     start=(kt == 0), stop=(kt == nkt - 1))
                    nc.tensor.matmul(pz[:], lhsT=lhsT,
                                     rhs=v_sb[:, kt, D : D + 1],
                                     start=(kt == 0), stop=(kt == nkt - 1))
                rec = wpool.tile([128, 1], F32, name="rec", bufs=2)
                nc.vector.reciprocal(rec[:], pz[:])
                o_sb = wpool.tile([128, D], F32, name="o_sb", bufs=3)
                nc.vector.tensor_scalar(
                    out=o_sb[:], in0=po[:], scalar1=rec[:], scalar2=None,
                    op0=AL.mult,
                )
                nc.sync.dma_start(out_d[jl * 128 : (jl + 1) * 128, :], o_sb[:])

    nc.compile()
    return nc


def _chunked(a):
    """[C*128, N] -> [128, C, N] contiguous."""
    c = a.shape[0] // 128
    return np.ascontiguousarray(a.reshape(c, 128, *a.shape[1:]).transpose(1, 0, 2))


def _qsel(h):
    """Global query rows handled by half h: interleaved 128-row q-tiles."""
    return np.concatenate(
        [np.arange(128 * (2 * jl + h), 128 * (2 * jl + h) + 128) for jl in range(NQT)]
    )


def build_in_maps(inputs):
    x = np.asarray(inputs["x"], dtype=np.float32)
    pad = np.asarray(inputs["pad_mask"])
    wq_h = _chunked(np.asarray(inputs["wq"], dtype=np.float32) * SW).astype(FP8NP)
    wk_h = _chunked(np.asarray(inputs["wk"], dtype=np.float32) * SW).astype(FP8NP)
    wv_h = _chunked(np.asarray(inputs["wv"], dtype=np.float32)).astype(BF16NP)

    in_maps = []
    for c in range(8):
        b, h = divmod(c, 2)
        qsel = _qsel(h)
        xt_c = _chunked(x[b].T)
        xtb = xt_c.astype(FP8NP)                           # [128, 8, 2048]
        xtb16 = xt_c.astype(BF16NP)
        xqb = _chunked(x[b, qsel, :].T).astype(FP8NP)      # [128, 8, 1024]
        keep = (~pad[b]).astype(np.float32)                     # [2048]
        padk = np.ascontiguousarray(keep.reshape(NKT, 128).T)   # [128, 16]
        # keep iff iota (= q_glob - 128h) >= thresh = 128*kt + p - 128*h
        thr = (
            np.add.outer(np.arange(128, dtype=np.float32),
                         128.0 * np.arange(NKT, dtype=np.float32))
            - np.float32(128 * h)
        ).astype(np.float32)                                    # [128, 16]
        in_maps.append({
            "xt": xtb, "xt16": xtb16, "xq": xqb, "wq": wq_h, "wk": wk_h,
            "wv": wv_h, "padk": padk, "thr": np.ascontiguousarray(thr),
        })
    return in_maps


def kernel(**inputs):
    global _NC_CACHE
    if _NC_CACHE is None:
        _NC_CACHE = _build_nc()
    nc = _NC_CACHE

    in_maps = build_in_maps(inputs)
    res = bass_utils.run_bass_kernel_spmd(nc, in_maps, core_ids=list(range(8)))
    out = np.empty((B, L, D), dtype=np.float32)
    for b in range(B):
        for h in range(2):
            out[b, _qsel(h)] = res.results[2 * b + h]["out"]
    return out
